# revision 1
# baseline (speedup 1.0000x reference)
"""Trainium2 Bass kernel for nn_LogisticRegressionPerStock.

Math:  h = sigmoid(einsum("bts,st->bs", x, W1) + b1);  out = h @ W2.T + b2
Shapes: x [1024, 24, 8192], W1 [8192, 24], W2 [8192, 8192].

Sharding: stock dim S is split across 8 cores (SLOC = 1024 each).
Core k computes h[:, sk] from x[:, :, sk] (DVE multiply-accumulate over T
with a host-prereplicated W1 broadcast tile, then sigmoid on ACT), PE-
transposes h chunks into hT [s-part, b-free] layout, then runs the GEMM
partial_k = h[:, sk] @ W2[:, sk].T as fp32r matmuls (full [B, S] output).
The host sums the 8 partials and adds b2.  No collectives needed.
"""

import sys

sys.path.insert(0, "/opt/trn_rl_repo")

import numpy as np

B, T, S = 1024, 24, 8192
NCORES = 8
SLOC = S // NCORES  # 1024 stocks per core
S_CHUNK = 512  # einsum free-dim chunk
P = 128

_compiled = {}


def _build_nc(b, t, sloc, s_out, s_chunk):
    import concourse.bass as bass
    import concourse.bacc as bacc
    import concourse.tile as tile
    from concourse import mybir

    f32 = mybir.dt.float32
    f32r = mybir.dt.float32r
    nb = b // P          # b tiles
    nsc = sloc // s_chunk  # einsum chunks
    njc = s_chunk // P   # 128-blocks per chunk (transposes)
    nk = sloc // P       # GEMM contraction tiles
    n_out = s_out // 512  # GEMM output chunks of 512

    nc = bacc.Bacc()
    x_d = nc.dram_tensor("x", [b, t, sloc], f32, kind="ExternalInput")
    # w1r: partition-replicated [128, 25, sloc]; rows 0..t-1 = W1.T slice,
    # row t = b1 slice (bias folded into the same broadcast tile).
    w1r_d = nc.dram_tensor("w1r", [P, t + 1, sloc], f32, kind="ExternalInput")
    w2t_d = nc.dram_tensor("w2t", [sloc, s_out], f32r, kind="ExternalInput")
    ident_d = nc.dram_tensor("ident", [P, P], f32, kind="ExternalInput")
    out_d = nc.dram_tensor("part", [b, s_out], f32, kind="ExternalOutput")

    with tile.TileContext(nc) as tc:
        with tc.tile_pool(name="persist", bufs=1) as pp:
            ident = pp.tile([P, P], f32)
            nc.sync.dma_start(ident[:], ident_d[:])
            hT = pp.tile([P, nk, b], f32r)  # hT[sp, k, b] = h[b, k*128+sp]

            # ---- Phase E: per-stock logistic regressions -> hT ----
            with (
                tc.tile_pool(name="xp", bufs=2) as xp,
                tc.tile_pool(name="w1p", bufs=1) as w1p,
                tc.tile_pool(name="ep", bufs=4) as ep,
                tc.tile_pool(name="et", bufs=4) as etp,
                tc.tile_pool(name="eps", bufs=4, space="PSUM") as epsp,
            ):
                for c in range(nsc):
                    w1bc = w1p.tile([P, t + 1, s_chunk], f32, tag="w1bc")
                    nc.sync.dma_start(
                        w1bc[:], w1r_d[:, :, c * s_chunk : (c + 1) * s_chunk]
                    )
                    for bt in range(nb):
                        xt = xp.tile([P, t, s_chunk], f32, tag="xt")
                        nc.sync.dma_start(
                            xt[:],
                            x_d[
                                bt * P : (bt + 1) * P,
                                :,
                                c * s_chunk : (c + 1) * s_chunk,
                            ],
                        )
                        acc = ep.tile([P, s_chunk], f32, tag="acc")
                        nc.vector.tensor_mul(acc[:], xt[:, 0, :], w1bc[:, 0, :])
                        for ti in range(1, t):
                            tmp = ep.tile([P, s_chunk], f32, tag="tmp")
                            nc.vector.tensor_mul(
                                tmp[:], xt[:, ti, :], w1bc[:, ti, :]
                            )
                            nc.vector.tensor_add(acc[:], acc[:], tmp[:])
                        nc.vector.tensor_add(acc[:], acc[:], w1bc[:, t, :])
                        hs = ep.tile([P, s_chunk], f32, tag="hs")
                        nc.scalar.activation(
                            hs[:], acc[:], mybir.ActivationFunctionType.Sigmoid
                        )
                        for j in range(njc):
                            ptile = epsp.tile([P, P], f32, tag="pt")
                            nc.tensor.transpose(
                                ptile[:], hs[:, j * P : (j + 1) * P], ident[:]
                            )
                            k = c * njc + j
                            nc.vector.tensor_copy(
                                hT[:, k, bt * P : (bt + 1) * P], ptile[:]
                            )

            # ---- Phase G: partial = h_local @ W2[:, sk].T  (fp32r) ----
            with (
                tc.tile_pool(name="w2p", bufs=2) as w2p,
                tc.tile_pool(name="op", bufs=4) as op,
                tc.tile_pool(name="gps", bufs=8, space="PSUM") as gpsp,
            ):
                for n in range(n_out):
                    w2tiles = []
                    for k in range(nk):
                        w2k = w2p.tile([P, 512], f32r, tag=f"w2_{k}")
                        nc.sync.dma_start(
                            w2k[:],
                            w2t_d[k * P : (k + 1) * P, n * 512 : (n + 1) * 512],
                        )
                        w2tiles.append(w2k)
                    for bt in range(nb):
                        ps = gpsp.tile([P, 512], f32, tag="ps")
                        for k in range(nk):
                            nc.tensor.matmul(
                                ps[:],
                                hT[:, k, bt * P : (bt + 1) * P],
                                w2tiles[k][:],
                                start=(k == 0),
                                stop=(k == nk - 1),
                            )
                        ot = op.tile([P, 512], f32, tag="ot")
                        nc.vector.tensor_copy(ot[:], ps[:])
                        nc.sync.dma_start(
                            out_d[bt * P : (bt + 1) * P, n * 512 : (n + 1) * 512],
                            ot[:],
                        )
    nc.finalize()
    return nc


def _get_nc():
    key = (B, T, SLOC, S, S_CHUNK)
    if key not in _compiled:
        _compiled[key] = _build_nc(B, T, SLOC, S, S_CHUNK)
    return _compiled[key]


def _host_prep(x, W1, b1, W2):
    W2T = np.ascontiguousarray(W2.T)  # [S_in, S_out]
    ident = np.eye(P, dtype=np.float32)
    in_maps = []
    for k in range(NCORES):
        sk = slice(k * SLOC, (k + 1) * SLOC)
        x_k = np.ascontiguousarray(x[:, :, sk])
        w1e = np.concatenate(
            [W1[sk].T, b1[sk][None, :]], axis=0
        )  # [T+1, SLOC]
        w1r = np.ascontiguousarray(
            np.broadcast_to(w1e[None], (P, T + 1, SLOC))
        ).astype(np.float32)
        w2t_k = W2T[sk]  # contiguous row-slice view [SLOC, S]
        in_maps.append(
            {"x": x_k, "w1r": w1r, "w2t": np.ascontiguousarray(w2t_k), "ident": ident}
        )
    return in_maps


def kernel(x, W1, b1, W2, b2):
    from concourse.bass_utils import run_bass_kernel_spmd

    nc = _get_nc()
    in_maps = _host_prep(
        np.asarray(x, dtype=np.float32),
        np.asarray(W1, dtype=np.float32),
        np.asarray(b1, dtype=np.float32),
        np.asarray(W2, dtype=np.float32),
    )
    res = run_bass_kernel_spmd(nc, in_maps, list(range(NCORES)))
    parts = [res.results[k]["part"] for k in range(NCORES)]
    out = parts[0].astype(np.float32)
    for p in parts[1:]:
        out += p
    out += np.asarray(b2, dtype=np.float32)[None, :]
    return out

